# revision 9
# baseline (speedup 1.0000x reference)
"""DGMNet forward kernel for Trainium2, 8-core data parallel.

Strategy: shard batch B across 8 cores. Per core, activations live
feature-major ([HID, batch_tile] in SBUF) so every matmul streams the
batch along the free dim with the tiny weights stationary (float32r,
1 cycle/row). The host pre-transposes x, appends t and a ones row
(which folds every bias into the matmuls), so the device never
transposes and never adds biases. Per recurrence step the xt
projections are recomputed into PSUM (accumulating matmul pairs)
instead of materializing U tiles. Z|G|R share one 3-bank PSUM tile so
a single wide tanh covers all three; H is consumed directly from PSUM
by the fused (G-1)*H op; the final projection row is DMA'd from PSUM
straight to DRAM. Wf_b is added on the host.
"""

import warnings

warnings.filterwarnings("ignore")

import numpy as np

B = 262144
XD = 100
HID = 128
N_CORES = 8
BS = B // N_CORES  # 32768 rows per core
KXT = XD + 2  # x features + t row + ones row
NT = 512  # batch columns per tile


def _build(n_steps: int, bs: int, nt: int):
    import concourse.bacc as bacc
    import concourse.tile as tile
    import concourse.mybir as mybir
    from contextlib import ExitStack

    f32 = mybir.dt.float32
    f32r = mybir.dt.float32r
    Tanh = mybir.ActivationFunctionType.Tanh
    Copy = mybir.ActivationFunctionType.Copy
    sub = mybir.AluOpType.subtract
    mult = mybir.AluOpType.mult

    ntiles = bs // nt
    nc = bacc.Bacc("TRN2", target_bir_lowering=False, debug=False,
                   num_devices=N_CORES)

    xt = nc.dram_tensor("xt", [KXT, bs], f32r, kind="ExternalInput").ap()
    w1 = nc.dram_tensor("w1", [KXT, 5 * HID], f32r, kind="ExternalInput").ap()
    w2 = nc.dram_tensor("w2", [HID, 4 * HID], f32r, kind="ExternalInput").ap()
    wf = nc.dram_tensor("wf", [HID, 1], f32r, kind="ExternalInput").ap()
    out = nc.dram_tensor("out", [1, bs], f32, kind="ExternalOutput").ap()

    with tile.TileContext(nc) as tc:
        with ExitStack() as ctx:
            consts = ctx.enter_context(tc.tile_pool(name="consts", bufs=1))
            xpool = ctx.enter_context(tc.tile_pool(name="xp", bufs=5))
            spool = ctx.enter_context(tc.tile_pool(name="sp", bufs=16))
            zpool = ctx.enter_context(tc.tile_pool(name="zp", bufs=6))
            epool = ctx.enter_context(tc.tile_pool(name="ep", bufs=6))
            psum = ctx.enter_context(
                tc.tile_pool(name="ps", bufs=2, space="PSUM"))

            w1_t = consts.tile([KXT, 5 * HID], f32r)
            nc.sync.dma_start(w1_t[:], w1[:, :])
            w2_t = consts.tile([HID, 4 * HID], f32r)
            nc.sync.dma_start(w2_t[:], w2[:, :])
            wf_t = consts.tile([HID, 1], f32r)
            nc.sync.dma_start(wf_t[:], wf[:, :])

            def w1s(k):
                return w1_t[:, k * HID:(k + 1) * HID]

            def w2s(k):
                return w2_t[:, k * HID:(k + 1) * HID]

            GRP = 4  # tiles software-pipelined together
            for j0 in range(0, ntiles, GRP):
                js = list(range(j0, min(j0 + GRP, ntiles)))
                xr = {}
                S = {}
                for j in js:
                    x_t = xpool.tile([KXT, nt], f32r, tag="x", name="x_t")
                    nc.sync.dma_start(x_t[:], xt[:, j * nt:(j + 1) * nt])
                    xr[j] = x_t[:]
                for j in js:
                    ps = psum.tile([HID, nt], f32, tag="h", name="ps")
                    nc.tensor.matmul(ps[:], w1s(0), xr[j],
                                     start=True, stop=True)
                    S[j] = spool.tile([HID, nt], f32r, tag="S", name="S1")
                    nc.scalar.activation(S[j][:], ps[:], Tanh)

                for _ in range(n_steps):
                    for j in js:
                        Sr = S[j][:]
                        pzgr = psum.tile([HID, 3 * nt], f32, tag="zgr", name="pzgr")
                        for k in range(3):
                            sl = pzgr[:, k * nt:(k + 1) * nt]
                            nc.tensor.matmul(sl, w1s(1 + k), xr[j],
                                             start=True, stop=False)
                            nc.tensor.matmul(sl, w2s(k), Sr,
                                             start=False, stop=True)
                        ZGR = zpool.tile([HID, 3 * nt], f32, tag="ZGR", name="ZGR")
                        nc.scalar.activation(ZGR[:], pzgr[:], Tanh)
                        Z = ZGR[:, 0:nt]
                        G = ZGR[:, nt:2 * nt]
                        R = ZGR[:, 2 * nt:3 * nt]

                        SR = epool.tile([HID, nt], f32r, tag="SR", name="SR")
                        nc.gpsimd.tensor_mul(SR[:], S[j][:], R)

                        ph = psum.tile([HID, nt], f32, tag="h", name="ph")
                        nc.tensor.matmul(ph[:], w1s(4), xr[j],
                                         start=True, stop=False)
                        nc.tensor.matmul(ph[:], w2s(3), SR[:],
                                         start=False, stop=True)

                        ZS = epool.tile([HID, nt], f32, tag="ZS", name="ZS")
                        nc.vector.tensor_mul(ZS[:], Z, S[j][:])
                        T1 = epool.tile([HID, nt], f32, tag="T1", name="T1")
                        nc.vector.scalar_tensor_tensor(T1[:], G, 1.0, ph[:],
                                                       op0=sub, op1=mult)
                        Snew = spool.tile([HID, nt], f32r, tag="S", name="Snew")
                        nc.vector.tensor_sub(Snew[:], ZS[:], T1[:])
                        S[j] = Snew

                for j in js:
                    po = psum.tile([1, nt], f32, tag="h", name="po")
                    nc.tensor.matmul(po[:], wf_t[:],
                                     S[j][:],
                                     start=True, stop=True)
                    o_t = xpool.tile([1, nt], f32, tag="o", name="o_t")
                    nc.scalar.activation(o_t[:], po[:], Copy)
                    nc.sync.dma_start(out[:, j * nt:(j + 1) * nt], o_t[:])

    nc.compile()
    return nc


_cache = {}


def _get_nc(n_steps: int, bs: int = BS, nt: int = NT):
    key = (n_steps, bs, nt)
    if key not in _cache:
        _cache[key] = _build(n_steps, bs, nt)
    return _cache[key]


def _pack_host(x, t, Sw_w, Sw_b, Uz_w, Uz_b, Wsz_w, Wsz_b, Ug_w, Ug_b, Wsg_w,
               Wsg_b, Ur_w, Ur_b, Wsr_w, Wsr_b, Uh_w, Uh_b, Wsh_w, Wsh_b,
               Wf_w):
    f32 = np.float32
    b_total = x.shape[0]
    xt_full = np.empty((KXT, b_total), dtype=f32)
    xt_full[:XD, :] = np.asarray(x, dtype=f32).T
    xt_full[XD, :] = np.asarray(t, dtype=f32)[:, 0]
    xt_full[XD + 1, :] = 1.0

    def blk(w, b):
        # [101, 128] weights + folded bias row -> [102, 128]
        return np.concatenate(
            [np.asarray(w, f32),
             np.asarray(b, f32).reshape(1, HID)], axis=0)

    w1 = np.concatenate([
        blk(Sw_w, Sw_b),
        blk(Uz_w, np.asarray(Uz_b) + np.asarray(Wsz_b)),
        blk(Ug_w, np.asarray(Ug_b) + np.asarray(Wsg_b)),
        blk(Ur_w, np.asarray(Ur_b) + np.asarray(Wsr_b)),
        blk(Uh_w, np.asarray(Uh_b) + np.asarray(Wsh_b)),
    ], axis=1).astype(f32)
    w2 = np.concatenate([Wsz_w, Wsg_w, Wsr_w, Wsh_w], axis=1).astype(f32)
    wf = np.asarray(Wf_w, dtype=f32).reshape(HID, 1)
    return xt_full, w1, w2, wf


def kernel(x, t, Sw_w, Sw_b, Uz_w, Uz_b, Wsz_w, Wsz_b, Ug_w, Ug_b, Wsg_w,
           Wsg_b, Ur_w, Ur_b, Wsr_w, Wsr_b, Uh_w, Uh_b, Wsh_w, Wsh_b, Wf_w,
           Wf_b, n_layers):
    from concourse.bass_utils import run_bass_kernel_spmd

    x = np.asarray(x)
    t = np.asarray(t)
    b_total = x.shape[0]
    assert b_total % N_CORES == 0
    bs = b_total // N_CORES
    n_steps = int(n_layers) - 1

    xt_full, w1, w2, wf = _pack_host(
        x, t, Sw_w, Sw_b, Uz_w, Uz_b, Wsz_w, Wsz_b, Ug_w, Ug_b, Wsg_w, Wsg_b,
        Ur_w, Ur_b, Wsr_w, Wsr_b, Uh_w, Uh_b, Wsh_w, Wsh_b, Wf_w)

    nc = _get_nc(n_steps, bs)

    in_maps = []
    for c in range(N_CORES):
        in_maps.append({
            "xt": np.ascontiguousarray(xt_full[:, c * bs:(c + 1) * bs]),
            "w1": w1,
            "w2": w2,
            "wf": wf,
        })

    res = run_bass_kernel_spmd(nc, in_maps, core_ids=list(range(N_CORES)))
    out = np.empty((b_total, 1), dtype=np.float32)
    bf = np.float32(np.asarray(Wf_b).reshape(-1)[0])
    for c in range(N_CORES):
        out[c * bs:(c + 1) * bs, 0] = res.results[c]["out"][0] + bf
    return out


# revision 16
# speedup vs baseline: 2.9925x; 2.9925x over previous
"""DGMNet forward kernel for Trainium2, 8-core data parallel.

Strategy: shard batch B across 8 cores. Per core, activations live
feature-major ([HID, batch_tile] in SBUF) so every matmul streams the
batch along the free dim with the tiny weights stationary (float32r,
1 cycle/row). The host pre-transposes x, appends t and a ones row
(which folds every bias into the matmuls), so the device never
transposes and never adds biases. Per recurrence step the xt
projections are recomputed into PSUM (accumulating matmul pairs)
instead of materializing U tiles. Z|G|R share one 3-bank PSUM tile so
a single wide tanh covers all three; H is consumed directly from PSUM
by the fused (G-1)*H op; the final projection row is DMA'd from PSUM
straight to DRAM. Wf_b is added on the host.
"""

import warnings

warnings.filterwarnings("ignore")

import numpy as np

B = 262144
XD = 100
HID = 128
N_CORES = 8
BS = B // N_CORES  # 32768 rows per core
KXT = XD + 2  # x features + t row + ones row
NT = 512  # batch columns per tile


def _build(n_steps: int, bs: int, nt: int, reps: int = 1, hw_loop: int = 0):
    import concourse.bacc as bacc
    import concourse.tile as tile
    import concourse.mybir as mybir
    from contextlib import ExitStack, nullcontext

    f32 = mybir.dt.float32
    f32r = mybir.dt.float32r
    Tanh = mybir.ActivationFunctionType.Tanh
    Copy = mybir.ActivationFunctionType.Copy
    sub = mybir.AluOpType.subtract
    mult = mybir.AluOpType.mult

    ntiles = bs // nt
    nc = bacc.Bacc("TRN2", target_bir_lowering=False, debug=False,
                   num_devices=N_CORES)

    xt = nc.dram_tensor("xt", [KXT, bs], f32r, kind="ExternalInput").ap()
    w1 = nc.dram_tensor("w1", [KXT, 5 * HID], f32r, kind="ExternalInput").ap()
    w2 = nc.dram_tensor("w2", [HID, 4 * HID], f32r, kind="ExternalInput").ap()
    wf = nc.dram_tensor("wf", [HID, 1], f32r, kind="ExternalInput").ap()
    out = nc.dram_tensor("out", [1, bs], f32, kind="ExternalOutput").ap()

    with tile.TileContext(nc) as tc:
        with ExitStack() as ctx:
            consts = ctx.enter_context(tc.tile_pool(name="consts", bufs=1))
            xpool = ctx.enter_context(tc.tile_pool(name="xp", bufs=7))
            spool = ctx.enter_context(tc.tile_pool(name="sp", bufs=20))
            zpool = ctx.enter_context(tc.tile_pool(name="zp", bufs=8))
            epool = ctx.enter_context(tc.tile_pool(name="ep", bufs=8))
            psum = ctx.enter_context(
                tc.tile_pool(name="ps", bufs=2, space="PSUM"))

            w1_t = consts.tile([KXT, 5 * HID], f32r)
            nc.sync.dma_start(w1_t[:], w1[:, :])
            w2_t = consts.tile([HID, 4 * HID], f32r)
            nc.sync.dma_start(w2_t[:], w2[:, :])
            wf_t = consts.tile([HID, 1], f32r)
            nc.sync.dma_start(wf_t[:], wf[:, :])

            def w1s(k):
                return w1_t[:, k * HID:(k + 1) * HID]

            def w2s(k):
                return w2_t[:, k * HID:(k + 1) * HID]

            # Optional HW loop repeating the whole pass (timing rig only)
            loop_cm = (tc.For_i(0, hw_loop, 1,
                                hint_engines=(mybir.EngineType.PE,
                                              mybir.EngineType.Activation,
                                              mybir.EngineType.DVE,
                                              mybir.EngineType.SP,
                                              mybir.EngineType.Pool))
                       if hw_loop else nullcontext())
            ctx.enter_context(loop_cm)

            GRP = 4  # tiles software-pipelined together
            for j0 in range(0, ntiles * reps, GRP):
                js = [jj % ntiles for jj in
                      range(j0, min(j0 + GRP, ntiles * reps))]
                xr = {}
                S = {}
                for j in js:
                    x_t = xpool.tile([KXT, nt], f32r, tag="x", name="x_t")
                    nc.sync.dma_start(x_t[:], xt[:, j * nt:(j + 1) * nt])
                    xr[j] = x_t[:]
                for j in js:
                    ps = psum.tile([HID, nt], f32, tag="h", name="ps")
                    nc.tensor.matmul(ps[:], w1s(0), xr[j],
                                     start=True, stop=True)
                    S[j] = spool.tile([HID, nt], f32r, tag="S", name="S1")
                    nc.scalar.activation(S[j][:], ps[:], Tanh)

                for _ in range(n_steps):
                    for j in js:
                        Sr = S[j][:]
                        pzgr = psum.tile([HID, 3 * nt], f32, tag="zgr", name="pzgr")
                        for k in range(3):
                            sl = pzgr[:, k * nt:(k + 1) * nt]
                            nc.tensor.matmul(sl, w1s(1 + k), xr[j],
                                             start=True, stop=False)
                            nc.tensor.matmul(sl, w2s(k), Sr,
                                             start=False, stop=True)
                        ZGR = zpool.tile([HID, 3 * nt], f32, tag="ZGR", name="ZGR")
                        nc.scalar.activation(ZGR[:], pzgr[:], Tanh)
                        Z = ZGR[:, 0:nt]
                        G = ZGR[:, nt:2 * nt]
                        R = ZGR[:, 2 * nt:3 * nt]

                        SR = epool.tile([HID, nt], f32r, tag="SR", name="SR")
                        nc.gpsimd.tensor_mul(SR[:], S[j][:], R)

                        ph = psum.tile([HID, nt], f32, tag="h", name="ph")
                        nc.tensor.matmul(ph[:], w1s(4), xr[j],
                                         start=True, stop=False)
                        nc.tensor.matmul(ph[:], w2s(3), SR[:],
                                         start=False, stop=True)

                        ZS = epool.tile([HID, nt], f32, tag="ZS", name="ZS")
                        nc.vector.tensor_mul(ZS[:], Z, S[j][:])
                        T1 = epool.tile([HID, nt], f32, tag="T1", name="T1")
                        nc.vector.scalar_tensor_tensor(T1[:], G, 1.0, ph[:],
                                                       op0=sub, op1=mult)
                        Snew = spool.tile([HID, nt], f32r, tag="S", name="Snew")
                        nc.vector.tensor_sub(Snew[:], ZS[:], T1[:])
                        S[j] = Snew

                for j in js:
                    po = psum.tile([1, nt], f32, tag="h", name="po")
                    nc.tensor.matmul(po[:], wf_t[:],
                                     S[j][:],
                                     start=True, stop=True)
                    o_t = xpool.tile([1, nt], f32, tag="o", name="o_t")
                    nc.vector.tensor_copy(o_t[:], po[:])
                    nc.sync.dma_start(out[:, j * nt:(j + 1) * nt], o_t[:])

    nc.compile()
    return nc


_cache = {}


def _get_nc(n_steps: int, bs: int = BS, nt: int = NT):
    key = (n_steps, bs, nt)
    if key not in _cache:
        _cache[key] = _build(n_steps, bs, nt)
    return _cache[key]


def _pack_host(x, t, Sw_w, Sw_b, Uz_w, Uz_b, Wsz_w, Wsz_b, Ug_w, Ug_b, Wsg_w,
               Wsg_b, Ur_w, Ur_b, Wsr_w, Wsr_b, Uh_w, Uh_b, Wsh_w, Wsh_b,
               Wf_w):
    f32 = np.float32
    b_total = x.shape[0]
    xt_full = np.empty((KXT, b_total), dtype=f32)
    xt_full[:XD, :] = np.asarray(x, dtype=f32).T
    xt_full[XD, :] = np.asarray(t, dtype=f32)[:, 0]
    xt_full[XD + 1, :] = 1.0

    def blk(w, b):
        # [101, 128] weights + folded bias row -> [102, 128]
        return np.concatenate(
            [np.asarray(w, f32),
             np.asarray(b, f32).reshape(1, HID)], axis=0)

    w1 = np.concatenate([
        blk(Sw_w, Sw_b),
        blk(Uz_w, np.asarray(Uz_b) + np.asarray(Wsz_b)),
        blk(Ug_w, np.asarray(Ug_b) + np.asarray(Wsg_b)),
        blk(Ur_w, np.asarray(Ur_b) + np.asarray(Wsr_b)),
        blk(Uh_w, np.asarray(Uh_b) + np.asarray(Wsh_b)),
    ], axis=1).astype(f32)
    w2 = np.concatenate([Wsz_w, Wsg_w, Wsr_w, Wsh_w], axis=1).astype(f32)
    wf = np.asarray(Wf_w, dtype=f32).reshape(HID, 1)
    return xt_full, w1, w2, wf


def kernel(x, t, Sw_w, Sw_b, Uz_w, Uz_b, Wsz_w, Wsz_b, Ug_w, Ug_b, Wsg_w,
           Wsg_b, Ur_w, Ur_b, Wsr_w, Wsr_b, Uh_w, Uh_b, Wsh_w, Wsh_b, Wf_w,
           Wf_b, n_layers):
    from concourse.bass_utils import run_bass_kernel_spmd

    x = np.asarray(x)
    t = np.asarray(t)
    b_total = x.shape[0]
    assert b_total % N_CORES == 0
    bs = b_total // N_CORES
    n_steps = int(n_layers) - 1

    xt_full, w1, w2, wf = _pack_host(
        x, t, Sw_w, Sw_b, Uz_w, Uz_b, Wsz_w, Wsz_b, Ug_w, Ug_b, Wsg_w, Wsg_b,
        Ur_w, Ur_b, Wsr_w, Wsr_b, Uh_w, Uh_b, Wsh_w, Wsh_b, Wf_w)

    nc = _get_nc(n_steps, bs)

    in_maps = []
    for c in range(N_CORES):
        in_maps.append({
            "xt": np.ascontiguousarray(xt_full[:, c * bs:(c + 1) * bs]),
            "w1": w1,
            "w2": w2,
            "wf": wf,
        })

    res = None
    for attempt in range(3):
        try:
            res = run_bass_kernel_spmd(nc, in_maps,
                                       core_ids=list(range(N_CORES)))
            break
        except Exception:
            if attempt == 2:
                raise
            import time as _time
            _time.sleep(5.0)
    out = np.empty((b_total, 1), dtype=np.float32)
    bf = np.float32(np.asarray(Wf_b).reshape(-1)[0])
    for c in range(N_CORES):
        out[c * bs:(c + 1) * bs, 0] = res.results[c]["out"][0] + bf
    return out
